# revision 9
# baseline (speedup 1.0000x reference)
"""Multi-head attention Trainium2 kernel (8 NeuronCores, SPMD).

Problem: nn_MultiHeadAttention (B=2, S=2048, D=768, H=12, d_k=64), f32 I/O.

Sharding: 24 (batch, head) pairs -> 8 cores x 3 heads. Core c handles
batch b = c // 4 and heads [3*(c%4), 3*(c%4)+3). Each core computes the
Q/K/V projections for its 3 heads, full-sequence attention, and its
partial contribution to the output projection. A 4-core ReduceScatter
(cores of the same batch) sums the partials and leaves each core with a
distinct 512-row slice of the batch output; the host concatenates.

v4 schedule. HW microbenchmarks (microbench.py): exp of a [128,2,512]
psum round = 788 ns (the optimum size; 512->503, 2048->1803), K=64
score-pair slot ~296 ns, K=128 N=512 accumulating matmul ~110-200 ns,
fp8 DoubleRow gives no real speedup, gpsimd DMA costs ~1us/transfer
(SWDGE). ACT is therefore the binding engine: 96 exp rounds = 75.6 us.
The schedule keeps ACT streaming:
  - per q-block iteration: 16 sc01 rounds (heads 0,1 paired) then 8 sc2
    rounds (head 2, chunk pairs on duplicated partitions). PV for heads
    0,1 trails its exps chunk-by-chunk (1-2 rounds behind); PV for head
    2 trails in rounds 17-24; norms right after; oproj of block qb runs
    in the first rounds of iteration qb+1.
  - PSUM: sc ring 2 ([128,2,512] x2 = 4 banks) + pv ring 2 + op ring 2.
    pv ring alloc order pv0,pv1,pv2 per iteration with frees at
    norm0/norm1 (round ~17) and norm2 (tail) keeps strict rotation legal.
  - all input DMA on the two HWDGE queues (sync: kt+qt column blocks,
    scalar: weights+vt); outputs + ReduceScatter on gpsimd so the next
    rep's input loads are never head-of-line blocked -> reps pipeline.
  - chunked prologue: k-proj block 0 + q-proj block 0 start as soon as
    ~1.6 MB has landed; remaining projections zip into iteration 0.
"""

import numpy as np
import ml_dtypes

B = 2
S = 2048
D = 768
H = 12
DK = 64
HPC = 3           # heads per core
HD = HPC * DK     # 192 head-feature columns per core
NCORES = 8
GROUP = 4         # cores per batch (reduce-scatter group)
QS = S // GROUP   # 512 output rows per core

_compiled = None
USE_PE_BCAST = False
USE_SC2_PAIR = True


def _build(reps=1, collective=True):
    """Build the SPMD program. reps>1 emits the whole pipeline N times
    back-to-back (same inputs/outputs) inside an on-device For_i loop —
    used for timing, where (T_reps - T_1)/(reps-1) cancels dispatch
    overhead. collective=False drops the final ReduceScatter."""
    import concourse.mybir as mybir
    import concourse.tile as tile
    from concourse import bacc
    from concourse.bass import ts
    import concourse.bass as bass

    bf16 = mybir.dt.bfloat16
    f32 = mybir.dt.float32

    nc = bacc.Bacc(num_devices=NCORES)

    qt = nc.dram_tensor("qt", [D, S], bf16, kind="ExternalInput")
    kt = nc.dram_tensor("kt", [D, S], bf16, kind="ExternalInput")
    vt = nc.dram_tensor("vt", [D, S], bf16, kind="ExternalInput")
    wq = nc.dram_tensor("wq", [D, HD], bf16, kind="ExternalInput")
    wk = nc.dram_tensor("wk", [D, HD], bf16, kind="ExternalInput")
    wv = nc.dram_tensor("wv", [D, HD], bf16, kind="ExternalInput")
    wo = nc.dram_tensor("wo", [HD + 1, D], bf16, kind="ExternalInput")
    bq = nc.dram_tensor("bq", [HD, 1], f32, kind="ExternalInput")
    bk = nc.dram_tensor("bk", [HD, 1], f32, kind="ExternalInput")
    bv = nc.dram_tensor("bv", [1, HD], f32, kind="ExternalInput")
    out_ext = nc.dram_tensor("out", [QS, D], bf16, kind="ExternalOutput")
    out_part = nc.dram_tensor("out_part", [S, D], bf16)
    out_rs = nc.dram_tensor("out_rs", [QS, D], bf16)

    RGROUPS = [list(range(g * GROUP, (g + 1) * GROUP))
               for g in range(NCORES // GROUP)]
    NC_ = D // 128      # 6 contraction chunks for the projections
    NKC = S // 128      # 16 kv chunks
    NQB = S // 512      # 4 q blocks
    VW = DK + 2         # 66-wide per-head V block: 64 dims + ones col + pad
    SCALE = float(1.0 / np.sqrt(DK))

    import contextlib

    with tile.TileContext(nc) as tc:
      with (tc.For_i(0, reps, 1) if reps > 1 else contextlib.nullcontext()):
       with contextlib.ExitStack() as ctx:
        consts = ctx.enter_context(tc.tile_pool(name="consts", bufs=1))
        acts = ctx.enter_context(tc.tile_pool(name="acts", bufs=1))

        # ---- input loads: two HWDGE queues only (SP carries kt+qt column
        # blocks in consumption order, ACT's queue carries weights + vt;
        # ACT pays only SEQ dispatch, the engine itself is untouched).
        # gpsimd (SWDGE, ~1us/transfer) is reserved for the output path so
        # rep i+1's input DMAs are never queued behind rep i's outputs.
        w_sb, bias_sb = {}, {}

        def load_w(name, t):
            sb = consts.tile([128, NC_, HD], bf16, tag=name)
            nc.scalar.dma_start(
                out=sb, in_=t[:, :].rearrange("(c p) n -> p c n", p=128))
            w_sb[name] = sb

        def load_bias(name, t):
            b0 = consts.tile([128, 1], f32, tag=name + "0")
            nc.scalar.dma_start(out=b0, in_=t[0:128, :])
            b1 = consts.tile([HD - 128, 1], f32, tag=name + "1")
            nc.scalar.dma_start(out=b1, in_=t[128:HD, :])
            bias_sb[name] = (b0, b1)

        # block-wise loads of the transposed activations:
        # x_sb[:, c, b*512:(b+1)*512] <- x[c*128:(c+1)*128, b*512:...]
        ins_sb = {}

        def load_block(name, t, b, q):
            sb = ins_sb.get(name + "t")
            if sb is None:
                sb = consts.tile([128, NC_, S], bf16, tag=name + "t")
                ins_sb[name + "t"] = sb
            for c in range(NC_):
                q.dma_start(out=sb[:, c, ts(b, 512)],
                            in_=t[c * 128:(c + 1) * 128, ts(b, 512)])

        # sync queue: kt b0, qt b0 (prologue-critical), then the rest
        load_w("wk", wk)          # scalar queue
        load_bias("bk", bk)
        load_block("k", kt, 0, nc.sync)
        load_w("wq", wq)
        load_bias("bq", bq)
        load_block("q", qt, 0, nc.sync)
        load_block("k", kt, 1, nc.sync)
        load_block("k", kt, 2, nc.sync)
        load_block("k", kt, 3, nc.sync)
        load_w("wv", wv)
        bv_bc = consts.tile([128, HD], f32, tag="bv")
        nc.scalar.dma_start(
            out=bv_bc,
            in_=bass.AP(tensor=bv[:, :].tensor, offset=bv[:, :].offset,
                        ap=[[0, 128]] + bv[:, :].ap[1:]))
        for b in range(NQB):
            load_block("v", vt, b, nc.scalar)
        wo0 = consts.tile([128, D], bf16, tag="wo0")
        nc.scalar.dma_start(out=wo0, in_=wo[0:128, :])
        wo1 = consts.tile([HD + 1 - 128, D], bf16, tag="wo1")
        nc.scalar.dma_start(out=wo1, in_=wo[128:HD + 1, :])
        for b in (1, 2, 3):
            load_block("q", qt, b, nc.sync)

        # Touch the exp table early so ACT's table DMA overlaps the loads.
        warm = consts.tile([1, 1], f32, tag="warm")
        nc.vector.memset(warm, 0.0)
        nc.scalar.activation(out=warm, in_=warm,
                             func=mybir.ActivationFunctionType.Exp)

        # ---- persistent SBUF activation tiles ----
        # heads 0,1 stacked [128, S]; head 2 duplicated on both halves
        qT0 = acts.tile([128, S], bf16, tag="qT0")
        kT0 = acts.tile([128, S], bf16, tag="kT0")
        qk1s = acts.tile([64, 2, S], bf16, tag="qk1s")
        qk0 = {"q": qT0, "k": kT0}
        outT0 = acts.tile([128, S], bf16, tag="outT0")
        outT1 = acts.tile([DK + 1, S], bf16, tag="outT1")
        nc.vector.memset(outT1[DK:DK + 1, :], 1.0)
        v_sb = acts.tile([128, NKC, HPC * VW], bf16, tag="v")
        for h in range(HPC):
            nc.vector.memset(v_sb[:, :, h * VW + DK:h * VW + DK + 1], 1.0)

        # PSUM (8 banks of [128,512]f32):
        #   sc: [128,2,512] x2 = 4 banks (score rounds)
        #   pv: [128,512] x2 = 2 banks (PV accumulators; alloc order
        #       pv0,pv1,pv2 per iteration, freed norm0/norm1/norm2)
        #   op: [128,512] x2 = 2 banks (projections + out-proj tiles)
        sc_pool = ctx.enter_context(
            tc.tile_pool(name="sc_psum", bufs=2, space="PSUM"))
        pv_pool = ctx.enter_context(
            tc.tile_pool(name="pv_psum", bufs=2, space="PSUM"))
        op_pool = ctx.enter_context(
            tc.tile_pool(name="op_psum", bufs=2, space="PSUM"))
        nrm_pool = ctx.enter_context(tc.tile_pool(name="nrm", bufs=4))
        fo_pool = ctx.enter_context(tc.tile_pool(name="fo", bufs=2))

        # exp tiles are persistent (reused every q block); the tile
        # framework tracks dependencies per AP range, so next-block exps
        # get WAR semaphores against this block's trailing PV reads
        e01_t = acts.tile([128, NKC, 2, 512], bf16, tag="e01")
        e2_t = acts.tile([128, NKC, 512], bf16, tag="e2")

        # ---- projection helpers ----
        def emit_proj_block(ps, name, b):
            x_sb = ins_sb[name + "t"]
            for c in range(NC_):
                nc.tensor.matmul(
                    ps, lhsT=w_sb["w" + name][:, c, 0:128],
                    rhs=x_sb[:, c, ts(b, 512)],
                    start=(c == 0), stop=(c == NC_ - 1))

        def evac_proj(ps, name, b):
            nc.vector.tensor_scalar_add(
                out=qk0[name][:, ts(b, 512)], in0=ps,
                scalar1=bias_sb["b" + name][0])

        def u_kproj(b):
            def emit():
                ps = op_pool.tile([128, 512], f32, tag="op", name="op")
                emit_proj_block(ps, "k", b)
                evac_proj(ps, "k", b)
            return emit

        def u_qproj(b):
            def emit():
                ps = op_pool.tile([128, 512], f32, tag="op", name="op")
                emit_proj_block(ps, "q", b)
                evac_proj(ps, "q", b)
            return emit

        def u_g1proj(b):
            # head 2 q/k for block b: M=64 pair on PE col groups (q rows
            # 0-63, k rows 64-127); the sc2 rounds column-pair within a
            # chunk, so no cross-half duplication is needed
            def emit():
                ps = op_pool.tile([128, 512], f32, tag="op", name="op")
                for c in range(NC_):
                    nc.tensor.matmul(
                        ps[0:64, :], lhsT=w_sb["wq"][:, c, 128:192],
                        rhs=ins_sb["qt"][:, c, ts(b, 512)],
                        start=(c == 0), stop=(c == NC_ - 1),
                        tile_position=(0, 0))
                    nc.tensor.matmul(
                        ps[64:128, :], lhsT=w_sb["wk"][:, c, 128:192],
                        rhs=ins_sb["kt"][:, c, ts(b, 512)],
                        start=(c == 0), stop=(c == NC_ - 1),
                        tile_position=(0, 64))
                nc.vector.tensor_scalar_add(
                    out=qk1s[:, 0, ts(b, 512)], in0=ps[0:64, :],
                    scalar1=bias_sb["bq"][1])
                nc.vector.tensor_scalar_add(
                    out=qk1s[:, 1, ts(b, 512)], in0=ps[64:128, :],
                    scalar1=bias_sb["bk"][1])
            return emit

        def u_vproj(st):
            def emit():
                ps = op_pool.tile([128, 512], f32, tag="op", name="op")
                for c in range(NC_):
                    nc.tensor.matmul(
                        ps[:, 0:HD], lhsT=ins_sb["vt"][:, c, ts(st, 128)],
                        rhs=w_sb["wv"][:, c, :],
                        start=(c == 0), stop=(c == NC_ - 1))
                for h in range(HPC):
                    nc.vector.tensor_add(
                        v_sb[:, st, h * VW:h * VW + DK],
                        ps[:, ts(h, 64)], bv_bc[:, ts(h, 64)])
            return emit

        # ---- score rounds ----
        def u_sc01(qb, c):
            def emit():
                ps = sc_pool.tile([128, 2, 512], f32, tag="sc", name="sc")
                for h in (0, 1):
                    nc.tensor.matmul(
                        ps[:, h, :],
                        lhsT=qk0["k"][ts(h, 64), ts(c, 128)],
                        rhs=qk0["q"][ts(h, 64), ts(qb, 512)],
                        start=True, stop=True)
                nc.scalar.activation(
                    out=e01_t[:, c, :, :], in_=ps[:, 0:2, :],
                    func=mybir.ActivationFunctionType.Exp, scale=SCALE)
            return emit

        def u_sc2(qb, p):
            # head 2, kv chunks 2p and 2p+1, each split into two M=64
            # column-paired matmuls at tile positions (0,0)/(0,64) — the
            # rhs (qT1) is shared, no partition duplication needed.
            # Output layout: ps[:, j, :] = scores of chunk 2p+j.
            def emit():
                ps = sc_pool.tile([128, 2, 512], f32, tag="sc", name="sc")
                for j in (0, 1):
                    kc = 2 * p + j
                    for half in (0, 1):
                        nc.tensor.matmul(
                            ps[ts(half, 64), j, :],
                            lhsT=qk1s[:, 1, kc * 128 + half * 64:
                                      kc * 128 + (half + 1) * 64],
                            rhs=qk1s[:, 0, ts(qb, 512)],
                            start=True, stop=True,
                            tile_position=(0, 64 * half))
                nc.scalar.activation(
                    out=e2_t[:, 2 * p:2 * p + 2, :], in_=ps[:, 0:2, :],
                    func=mybir.ActivationFunctionType.Exp, scale=SCALE)
            return emit

        # ---- PV: chunk-granular accumulation, trailing the exps.
        # pv ring-2 alloc order: pv0(qb)@s2, pv1(qb)@tail, pv0(qb+1)@s2,
        # pv2(qb)@s10 of it(qb+1), ... with frees norm0@s18, norm1@s7,
        # norm2@s18 keeping strict rotation legal (verified on paper).
        pvt = {}         # (h, qb) -> PV PSUM tile

        def u_pv(h, qb, chunks):
            # accumulate the given kv chunks of head h into pvt[(h, qb)]
            def emit():
                if chunks[0] == 0:
                    pvt[(h, qb)] = pv_pool.tile([128, 512], f32, tag="pv",
                                                name="pv")
                ps = pvt[(h, qb)]
                for kc in chunks:
                    rhs = (e01_t[:, kc, h, :] if h < 2
                           else e2_t[:, kc, :])
                    nc.tensor.matmul(
                        ps[0:DK + 1, :],
                        lhsT=v_sb[:, kc, h * VW:h * VW + DK + 1],
                        rhs=rhs,
                        start=(kc == 0), stop=(kc == NKC - 1))
            return emit

        ones64 = consts.tile([1, 64], bf16, tag="ones64")
        nc.vector.memset(ones64, 1.0)

        def u_norm(h, qb):
            # reciprocal of the ones-row -> K=1 PE outer product with a
            # ones row broadcasts it across 64 partitions (PE has slack,
            # gpsimd does not) -> DVE multiply into outT.
            def emit():
                ps = pvt[(h, qb)]
                recip = nrm_pool.tile([1, 512], bf16, tag="recip",
                                      name="recip")
                with nc.allow_low_precision(reason="softmax recip in bf16"):
                    nc.vector.reciprocal(recip, ps[DK:DK + 1, :])
                if USE_PE_BCAST:
                    rbc = op_pool.tile([128, 512], f32, tag="op", name="op")
                    nc.tensor.matmul(rbc[0:64, :], lhsT=ones64, rhs=recip,
                                     start=True, stop=True)
                    rbc = rbc[0:64, :]
                else:
                    rbc = nrm_pool.tile([64, 512], bf16, tag="rbc",
                                        name="rbc")
                    nc.gpsimd.partition_broadcast(rbc, recip)
                dst = (outT0[ts(h, 64), ts(qb, 512)] if h < 2
                       else outT1[0:64, ts(qb, 512)])
                nc.vector.tensor_mul(dst, ps[0:DK, :], rbc)
            return emit

        ot_tile = {}

        def u_oproj(qb, sq):
            # output projection for q sub-tile qb*4+sq, N split 512+256
            def emit():
                if sq == 0:
                    ot_tile[qb] = fo_pool.tile([128, 4, D], bf16, tag="ot",
                                               name="ot")
                qt_ = qb * 4 + sq
                for noff, nsz in ((0, 512), (512, 256)):
                    ps = op_pool.tile([128, 512], f32, tag="op", name="op")
                    nc.tensor.matmul(
                        ps[:, 0:nsz], lhsT=outT0[:, ts(qt_, 128)],
                        rhs=wo0[:, noff:noff + nsz], start=True, stop=False)
                    nc.tensor.matmul(
                        ps[:, 0:nsz], lhsT=outT1[:, ts(qt_, 128)],
                        rhs=wo1[:, noff:noff + nsz], start=False, stop=True)
                    nc.vector.tensor_copy(
                        out=ot_tile[qb][:, sq, noff:noff + nsz],
                        in_=ps[:, 0:nsz])
            return emit

        def u_out(qb):
            # output DMA + collective on the gpsimd queue only
            def emit():
                nc.gpsimd.dma_start(
                    out=out_part[:, :].rearrange(
                        "(g t p) d -> g p t d", p=128, t=4)[qb],
                    in_=ot_tile[qb])
                if collective:
                    nc.gpsimd.collective_compute(
                        "ReduceScatter", mybir.AluOpType.add,
                        replica_groups=RGROUPS,
                        ins=[out_part[ts(qb, 512), :]],
                        outs=[out_rs[ts(qb, 128), :]])
                nc.gpsimd.dma_start(out=out_ext[ts(qb, 128), :],
                                    in_=(out_rs if collective
                                         else out_part)[ts(qb, 128), :])
            return emit

        def zip_emit(rounds, sec):
            """One ACT-gated score round, then the secondary units mapped
            to that slot; keys beyond len(rounds) form the tail."""
            for i, r in enumerate(rounds):
                r()
                for u in sec.get(i, ()):
                    u()
            for i in sorted(k for k in sec if k >= len(rounds)):
                for u in sec[i]:
                    u()

        def at(sec, i, u):
            sec.setdefault(i, []).append(u)

        # ---- prologue: k-proj block 0 + q-proj block 0 ----
        u_kproj(0)()
        u_qproj(0)()

        # ---- pipeline over q blocks ----
        # steady-state it(qb): rounds r0-15 sc01(qb,c=r), r16-23
        # sc2(qb,p=r-16). Secondary slots (see docstring):
        #   s0-6   pv1(qb-1) chunk pairs (2s+2, 2s+3)
        #   s c+2  pv0(qb) chunk c           (it0: s c+6, after vproj)
        #   s7     norm1(qb-1)  [+qp2/qp3 in it1/it2]
        #   s10-17 pv2(qb-1) pair p
        #   s18    norm0(qb), norm2(qb-1)
        #   s20-23 oproj(qb-1)
        #   tail   out(qb-1), pv1(qb) alloc + chunks 0,1
        for qb in range(NQB):
            rounds = ([u_sc01(qb, c) for c in range(NKC)] +
                      [u_sc2(qb, p) for p in range(NKC // 2)])
            sec = {}
            if qb == 0:
                for i, u in enumerate([u_kproj(1), u_g1proj(0), u_kproj(2),
                                       u_g1proj(1), u_kproj(3), u_g1proj(2),
                                       u_g1proj(3), u_qproj(1)]):
                    at(sec, i, u)
                for st in range(NKC):
                    at(sec, st + 5, u_vproj(st))
                for c in range(NKC):
                    at(sec, c + 6, u_pv(0, qb, (c,)))
                at(sec, 22, u_norm(0, qb))
            else:
                for s in range(7):
                    at(sec, s, u_pv(1, qb - 1, (2 * s + 2, 2 * s + 3)))
                at(sec, 7, u_norm(1, qb - 1))
                if qb == 1:
                    at(sec, 8, u_qproj(2))
                if qb == 2:
                    at(sec, 8, u_qproj(3))
                for c in range(NKC):
                    at(sec, c + 2, u_pv(0, qb, (c,)))
                for p in range(NKC // 2):
                    at(sec, 10 + p, u_pv(2, qb - 1, (2 * p, 2 * p + 1)))
                at(sec, 18, u_norm(0, qb))
                at(sec, 18, u_norm(2, qb - 1))
                for sq in range(4):
                    at(sec, 20 + sq, u_oproj(qb - 1, sq))
                at(sec, 24, u_out(qb - 1))
            # pv1(qb) first pair in the tail (before it(qb+1)'s round 0
            # overwrites e01 chunk 0)
            at(sec, 24, u_pv(1, qb, (0, 1)))
            zip_emit(rounds, sec)

        # ---- post: drain qb3 ----
        qb = NQB - 1
        for s in range(7):
            u_pv(1, qb, (2 * s + 2, 2 * s + 3))()
        u_norm(1, qb)()
        for p in range(NKC // 2):
            u_pv(2, qb, (2 * p, 2 * p + 1))()
        u_norm(2, qb)()
        for sq in range(4):
            u_oproj(qb, sq)()
        u_out(qb)()

    nc.compile()
    return nc


def _get_compiled():
    global _compiled
    if _compiled is None:
        _compiled = _build()
    return _compiled


def make_in_maps(q, k, v, Wq, bq, Wk, bk, Wv, bv, Wo, bo):
    bf = ml_dtypes.bfloat16
    in_maps = []
    for c in range(NCORES):
        b = c // GROUP
        g = c % GROUP
        cols = slice(g * HD, (g + 1) * HD)   # head-feature columns
        wo_aug = np.empty((HD + 1, D), np.float32)
        wo_aug[:HD] = Wo.T[cols.start:cols.stop, :]
        wo_aug[HD] = bo / GROUP              # summed GROUP times by the RS
        in_maps.append({
            "qt": np.ascontiguousarray(q[b].T).astype(bf),
            "kt": np.ascontiguousarray(k[b].T).astype(bf),
            "vt": np.ascontiguousarray(v[b].T).astype(bf),
            "wq": np.ascontiguousarray(Wq.T[:, cols]).astype(bf),
            "wk": np.ascontiguousarray(Wk.T[:, cols]).astype(bf),
            "wv": np.ascontiguousarray(Wv.T[:, cols]).astype(bf),
            "wo": wo_aug.astype(bf),
            "bq": np.ascontiguousarray(bq[cols].reshape(HD, 1)).astype(np.float32),
            "bk": np.ascontiguousarray(bk[cols].reshape(HD, 1)).astype(np.float32),
            "bv": np.ascontiguousarray(bv[cols].reshape(1, HD)).astype(np.float32),
        })
    return in_maps


def kernel(q, k, v, Wq, bq, Wk, bk, Wv, bv, Wo, bo):
    from concourse.bass_utils import run_bass_kernel_spmd

    q = np.asarray(q, np.float32)
    k = np.asarray(k, np.float32)
    v = np.asarray(v, np.float32)
    nc = _get_compiled()
    in_maps = make_in_maps(q, k, v,
                           np.asarray(Wq, np.float32), np.asarray(bq, np.float32),
                           np.asarray(Wk, np.float32), np.asarray(bk, np.float32),
                           np.asarray(Wv, np.float32), np.asarray(bv, np.float32),
                           np.asarray(Wo, np.float32), np.asarray(bo, np.float32))
    res = run_bass_kernel_spmd(nc, in_maps, list(range(NCORES))).results
    out = np.empty((B, S, D), np.float32)
    for c in range(NCORES):
        b = c // GROUP
        j = c % GROUP
        # chunked reduce-scatter: chunk g of core (b, j) holds batch-b
        # rows [512*g + 128*j, 512*g + 128*j + 128)
        chunks = res[c]["out"].reshape(GROUP, 128, D)
        for g in range(GROUP):
            out[b, 512 * g + 128 * j:512 * g + 128 * j + 128, :] = chunks[g]
    return out


# revision 11
# speedup vs baseline: 1.0719x; 1.0719x over previous
"""Multi-head attention Trainium2 kernel (8 NeuronCores, SPMD).

Problem: nn_MultiHeadAttention (B=2, S=2048, D=768, H=12, d_k=64), f32 I/O.

Sharding: 24 (batch, head) pairs -> 8 cores x 3 heads. Core c handles
batch b = c // 4 and heads [3*(c%4), 3*(c%4)+3). Each core computes the
Q/K/V projections for its 3 heads, full-sequence attention, and its
partial contribution to the output projection. A 4-core ReduceScatter
(cores of the same batch) sums the partials and leaves each core with a
distinct 512-row slice of the batch output; the host concatenates.

v4 schedule. HW microbenchmarks (microbench.py): exp of a [128,2,512]
psum round = 788 ns (the optimum size; 512->503, 2048->1803), K=64
score-pair slot ~296 ns, K=128 N=512 accumulating matmul ~110-200 ns,
fp8 DoubleRow gives no real speedup, gpsimd DMA costs ~1us/transfer
(SWDGE). ACT is therefore the binding engine: 96 exp rounds = 75.6 us.
The schedule keeps ACT streaming:
  - per q-block iteration: 16 sc01 rounds (heads 0,1 paired) then 8 sc2
    rounds (head 2, chunk pairs on duplicated partitions). PV for heads
    0,1 trails its exps chunk-by-chunk (1-2 rounds behind); PV for head
    2 trails in rounds 17-24; norms right after; oproj of block qb runs
    in the first rounds of iteration qb+1.
  - PSUM: sc ring 2 ([128,2,512] x2 = 4 banks) + pv ring 2 + op ring 2.
    pv ring alloc order pv0,pv1,pv2 per iteration with frees at
    norm0/norm1 (round ~17) and norm2 (tail) keeps strict rotation legal.
  - all input DMA on the two HWDGE queues (sync: kt+qt column blocks,
    scalar: weights+vt); outputs + ReduceScatter on gpsimd so the next
    rep's input loads are never head-of-line blocked -> reps pipeline.
  - chunked prologue: k-proj block 0 + q-proj block 0 start as soon as
    ~1.6 MB has landed; remaining projections zip into iteration 0.
"""

import numpy as np
import ml_dtypes

B = 2
S = 2048
D = 768
H = 12
DK = 64
HPC = 3           # heads per core
HD = HPC * DK     # 192 head-feature columns per core
NCORES = 8
GROUP = 4         # cores per batch (reduce-scatter group)
QS = S // GROUP   # 512 output rows per core

_compiled = None
USE_PE_BCAST = False
USE_SC2_PAIR = True


def _build(reps=1, collective=True):
    """Build the SPMD program. reps>1 emits the whole pipeline N times
    back-to-back (same inputs/outputs) inside an on-device For_i loop —
    used for timing, where (T_reps - T_1)/(reps-1) cancels dispatch
    overhead. collective=False drops the final ReduceScatter."""
    import concourse.mybir as mybir
    import concourse.tile as tile
    from concourse import bacc
    from concourse.bass import ts
    import concourse.bass as bass

    bf16 = mybir.dt.bfloat16
    f32 = mybir.dt.float32

    nc = bacc.Bacc(num_devices=NCORES)

    qt = nc.dram_tensor("qt", [D, S], bf16, kind="ExternalInput")
    kt = nc.dram_tensor("kt", [D, S], bf16, kind="ExternalInput")
    vt = nc.dram_tensor("vt", [D, S], bf16, kind="ExternalInput")
    wq = nc.dram_tensor("wq", [D, HD], bf16, kind="ExternalInput")
    wk = nc.dram_tensor("wk", [D, HD], bf16, kind="ExternalInput")
    wv = nc.dram_tensor("wv", [D, HD], bf16, kind="ExternalInput")
    wo = nc.dram_tensor("wo", [HD + 1, D], bf16, kind="ExternalInput")
    bq = nc.dram_tensor("bq", [HD, 1], f32, kind="ExternalInput")
    bk = nc.dram_tensor("bk", [HD, 1], f32, kind="ExternalInput")
    bv = nc.dram_tensor("bv", [1, HD], f32, kind="ExternalInput")
    out_ext = nc.dram_tensor("out", [QS, D], bf16, kind="ExternalOutput")
    out_part = nc.dram_tensor("out_part", [S, D], bf16)
    out_rs = nc.dram_tensor("out_rs", [QS, D], bf16)

    RGROUPS = [list(range(g * GROUP, (g + 1) * GROUP))
               for g in range(NCORES // GROUP)]
    NC_ = D // 128      # 6 contraction chunks for the projections
    NKC = S // 128      # 16 kv chunks
    NQB = S // 512      # 4 q blocks
    VW = DK + 2         # 66-wide per-head V block: 64 dims + ones col + pad
    SCALE = float(1.0 / np.sqrt(DK))

    import contextlib

    with tile.TileContext(nc) as tc:
      with (tc.For_i(0, reps, 1) if reps > 1 else contextlib.nullcontext()):
       with contextlib.ExitStack() as ctx:
        consts = ctx.enter_context(tc.tile_pool(name="consts", bufs=1))
        acts = ctx.enter_context(tc.tile_pool(name="acts", bufs=1))

        # ---- input loads: DMA dispatch costs ~0.6-4.5us of the issuing
        # queue's SEQ per transfer (cost model + trace), so the scalar/ACT
        # queue must carry NO bulk input DMA (it would block the exps).
        # Use few, large, full-row transfers split across sync (HWDGE) and
        # gpsimd (SWDGE); the many-descriptor rearranged weight loads go
        # on gpsimd where dispatch is ~1.25us.
        w_sb, bias_sb = {}, {}
        ins_sb = {}

        def load_w(name, t):
            sb = consts.tile([128, NC_, HD], bf16, tag=name)
            nc.gpsimd.dma_start(
                out=sb, in_=t[:, :].rearrange("(c p) n -> p c n", p=128))
            w_sb[name] = sb

        def load_bias(name, t):
            b0 = consts.tile([128, 1], f32, tag=name + "0")
            nc.sync.dma_start(out=b0, in_=t[0:128, :])
            b1 = consts.tile([HD - 128, 1], f32, tag=name + "1")
            nc.sync.dma_start(out=b1, in_=t[128:HD, :])
            bias_sb[name] = (b0, b1)

        def load_rows(name, t, cs, q):
            sb = ins_sb.get(name + "t")
            if sb is None:
                sb = consts.tile([128, NC_, S], bf16, tag=name + "t")
                ins_sb[name + "t"] = sb
            for c in cs:
                q.dma_start(out=sb[:, c, :],
                            in_=t[c * 128:(c + 1) * 128, :])

        load_w("wk", wk)                      # gpsimd, first
        load_rows("k", kt, (0, 2, 4), nc.sync)
        load_rows("k", kt, (1, 3, 5), nc.gpsimd)
        load_bias("bk", bk)
        load_bias("bq", bq)
        load_w("wq", wq)
        load_rows("q", qt, (0, 2, 4), nc.sync)
        load_rows("q", qt, (1, 3, 5), nc.gpsimd)
        load_w("wv", wv)
        load_rows("v", vt, (0, 2, 4), nc.sync)
        load_rows("v", vt, (1, 3, 5), nc.gpsimd)
        bv_bc = consts.tile([128, HD], f32, tag="bv")
        nc.sync.dma_start(
            out=bv_bc,
            in_=bass.AP(tensor=bv[:, :].tensor, offset=bv[:, :].offset,
                        ap=[[0, 128]] + bv[:, :].ap[1:]))
        wo0 = consts.tile([128, D], bf16, tag="wo0")
        nc.sync.dma_start(out=wo0, in_=wo[0:128, :])
        wo1 = consts.tile([HD + 1 - 128, D], bf16, tag="wo1")
        nc.sync.dma_start(out=wo1, in_=wo[128:HD + 1, :])

        # Touch the exp table early so ACT's table DMA overlaps the loads.
        warm = consts.tile([1, 1], f32, tag="warm")
        nc.vector.memset(warm, 0.0)
        nc.scalar.activation(out=warm, in_=warm,
                             func=mybir.ActivationFunctionType.Exp)

        # ---- persistent SBUF activation tiles ----
        # heads 0,1 stacked [128, S]; head 2 duplicated on both halves
        qT0 = acts.tile([128, S], bf16, tag="qT0")
        kT0 = acts.tile([128, S], bf16, tag="kT0")
        qk1s = acts.tile([64, 2, S], bf16, tag="qk1s")
        qk0 = {"q": qT0, "k": kT0}
        outT0 = acts.tile([128, S], bf16, tag="outT0")
        outT1 = acts.tile([DK + 1, S], bf16, tag="outT1")
        nc.vector.memset(outT1[DK:DK + 1, :], 1.0)
        v_sb = acts.tile([128, NKC, HPC * VW], bf16, tag="v")
        for h in range(HPC):
            nc.vector.memset(v_sb[:, :, h * VW + DK:h * VW + DK + 1], 1.0)

        # PSUM (8 banks of [128,512]f32):
        #   sc: [128,2,512] x2 = 4 banks (score rounds)
        #   pv: [128,512] x2 = 2 banks (PV accumulators; alloc order
        #       pv0,pv1,pv2 per iteration, freed norm0/norm1/norm2)
        #   op: [128,512] x2 = 2 banks (projections + out-proj tiles)
        sc_pool = ctx.enter_context(
            tc.tile_pool(name="sc_psum", bufs=2, space="PSUM"))
        pv_pool = ctx.enter_context(
            tc.tile_pool(name="pv_psum", bufs=2, space="PSUM"))
        op_pool = ctx.enter_context(
            tc.tile_pool(name="op_psum", bufs=2, space="PSUM"))
        nrm_pool = ctx.enter_context(tc.tile_pool(name="nrm", bufs=4))
        fo_pool = ctx.enter_context(tc.tile_pool(name="fo", bufs=2))

        # exp tiles are persistent (reused every q block); the tile
        # framework tracks dependencies per AP range, so next-block exps
        # get WAR semaphores against this block's trailing PV reads
        e01_t = acts.tile([128, NKC, 2, 512], bf16, tag="e01")
        e2_t = acts.tile([128, NKC, 512], bf16, tag="e2")

        # ---- projection helpers ----
        def emit_proj_block(ps, name, b):
            x_sb = ins_sb[name + "t"]
            for c in range(NC_):
                nc.tensor.matmul(
                    ps, lhsT=w_sb["w" + name][:, c, 0:128],
                    rhs=x_sb[:, c, ts(b, 512)],
                    start=(c == 0), stop=(c == NC_ - 1))

        def evac_proj(ps, name, b):
            nc.vector.tensor_scalar_add(
                out=qk0[name][:, ts(b, 512)], in0=ps,
                scalar1=bias_sb["b" + name][0])

        def u_kproj(b):
            def emit():
                ps = op_pool.tile([128, 512], f32, tag="op", name="op")
                emit_proj_block(ps, "k", b)
                evac_proj(ps, "k", b)
            return emit

        def u_qproj(b):
            def emit():
                ps = op_pool.tile([128, 512], f32, tag="op", name="op")
                emit_proj_block(ps, "q", b)
                evac_proj(ps, "q", b)
            return emit

        def u_g1proj(b):
            # head 2 q/k for block b: M=64 pair on PE col groups (q rows
            # 0-63, k rows 64-127); the sc2 rounds column-pair within a
            # chunk, so no cross-half duplication is needed
            def emit():
                ps = op_pool.tile([128, 512], f32, tag="op", name="op")
                for c in range(NC_):
                    nc.tensor.matmul(
                        ps[0:64, :], lhsT=w_sb["wq"][:, c, 128:192],
                        rhs=ins_sb["qt"][:, c, ts(b, 512)],
                        start=(c == 0), stop=(c == NC_ - 1),
                        tile_position=(0, 0))
                    nc.tensor.matmul(
                        ps[64:128, :], lhsT=w_sb["wk"][:, c, 128:192],
                        rhs=ins_sb["kt"][:, c, ts(b, 512)],
                        start=(c == 0), stop=(c == NC_ - 1),
                        tile_position=(0, 64))
                nc.vector.tensor_scalar_add(
                    out=qk1s[:, 0, ts(b, 512)], in0=ps[0:64, :],
                    scalar1=bias_sb["bq"][1])
                nc.vector.tensor_scalar_add(
                    out=qk1s[:, 1, ts(b, 512)], in0=ps[64:128, :],
                    scalar1=bias_sb["bk"][1])
            return emit

        def u_vproj(st):
            def emit():
                ps = op_pool.tile([128, 512], f32, tag="op", name="op")
                for c in range(NC_):
                    nc.tensor.matmul(
                        ps[:, 0:HD], lhsT=ins_sb["vt"][:, c, ts(st, 128)],
                        rhs=w_sb["wv"][:, c, :],
                        start=(c == 0), stop=(c == NC_ - 1))
                for h in range(HPC):
                    nc.vector.tensor_add(
                        v_sb[:, st, h * VW:h * VW + DK],
                        ps[:, ts(h, 64)], bv_bc[:, ts(h, 64)])
            return emit

        # ---- score rounds ----
        def u_sc01(qb, c):
            def emit():
                ps = sc_pool.tile([128, 2, 512], f32, tag="sc", name="sc")
                for h in (0, 1):
                    nc.tensor.matmul(
                        ps[:, h, :],
                        lhsT=qk0["k"][ts(h, 64), ts(c, 128)],
                        rhs=qk0["q"][ts(h, 64), ts(qb, 512)],
                        start=True, stop=True)
                nc.scalar.activation(
                    out=e01_t[:, c, :, :], in_=ps[:, 0:2, :],
                    func=mybir.ActivationFunctionType.Exp, scale=SCALE)
            return emit

        def u_sc2(qb, p):
            # head 2, kv chunks 2p and 2p+1, each split into two M=64
            # column-paired matmuls at tile positions (0,0)/(0,64) — the
            # rhs (qT1) is shared, no partition duplication needed.
            # Output layout: ps[:, j, :] = scores of chunk 2p+j.
            def emit():
                ps = sc_pool.tile([128, 2, 512], f32, tag="sc", name="sc")
                for j in (0, 1):
                    kc = 2 * p + j
                    for half in (0, 1):
                        nc.tensor.matmul(
                            ps[ts(half, 64), j, :],
                            lhsT=qk1s[:, 1, kc * 128 + half * 64:
                                      kc * 128 + (half + 1) * 64],
                            rhs=qk1s[:, 0, ts(qb, 512)],
                            start=True, stop=True,
                            tile_position=(0, 64 * half))
                nc.scalar.activation(
                    out=e2_t[:, 2 * p:2 * p + 2, :], in_=ps[:, 0:2, :],
                    func=mybir.ActivationFunctionType.Exp, scale=SCALE)
            return emit

        # ---- PV: chunk-granular accumulation, trailing the exps.
        # pv ring-2 alloc order: pv0(qb)@s2, pv1(qb)@tail, pv0(qb+1)@s2,
        # pv2(qb)@s10 of it(qb+1), ... with frees norm0@s18, norm1@s7,
        # norm2@s18 keeping strict rotation legal (verified on paper).
        pvt = {}         # (h, qb) -> PV PSUM tile

        def u_pv(h, qb, chunks):
            # accumulate the given kv chunks of head h into pvt[(h, qb)]
            def emit():
                if chunks[0] == 0:
                    pvt[(h, qb)] = pv_pool.tile([128, 512], f32, tag="pv",
                                                name="pv")
                ps = pvt[(h, qb)]
                for kc in chunks:
                    rhs = (e01_t[:, kc, h, :] if h < 2
                           else e2_t[:, kc, :])
                    nc.tensor.matmul(
                        ps[0:DK + 1, :],
                        lhsT=v_sb[:, kc, h * VW:h * VW + DK + 1],
                        rhs=rhs,
                        start=(kc == 0), stop=(kc == NKC - 1))
            return emit

        ones64 = consts.tile([1, 64], bf16, tag="ones64")
        nc.vector.memset(ones64, 1.0)

        def u_norm(h, qb):
            # reciprocal of the ones-row -> K=1 PE outer product with a
            # ones row broadcasts it across 64 partitions (PE has slack,
            # gpsimd does not) -> DVE multiply into outT.
            def emit():
                ps = pvt[(h, qb)]
                recip = nrm_pool.tile([1, 512], bf16, tag="recip",
                                      name="recip")
                with nc.allow_low_precision(reason="softmax recip in bf16"):
                    nc.vector.reciprocal(recip, ps[DK:DK + 1, :])
                if USE_PE_BCAST:
                    rbc = op_pool.tile([128, 512], f32, tag="op", name="op")
                    nc.tensor.matmul(rbc[0:64, :], lhsT=ones64, rhs=recip,
                                     start=True, stop=True)
                    rbc = rbc[0:64, :]
                else:
                    rbc = nrm_pool.tile([64, 512], bf16, tag="rbc",
                                        name="rbc")
                    nc.gpsimd.partition_broadcast(rbc, recip)
                dst = (outT0[ts(h, 64), ts(qb, 512)] if h < 2
                       else outT1[0:64, ts(qb, 512)])
                nc.vector.tensor_mul(dst, ps[0:DK, :], rbc)
            return emit

        ot_tile = {}

        def u_oproj(qb, sq):
            # output projection for q sub-tile qb*4+sq, N split 512+256
            def emit():
                if sq == 0:
                    ot_tile[qb] = fo_pool.tile([128, 4, D], bf16, tag="ot",
                                               name="ot")
                qt_ = qb * 4 + sq
                for noff, nsz in ((0, 512), (512, 256)):
                    ps = op_pool.tile([128, 512], f32, tag="op", name="op")
                    nc.tensor.matmul(
                        ps[:, 0:nsz], lhsT=outT0[:, ts(qt_, 128)],
                        rhs=wo0[:, noff:noff + nsz], start=True, stop=False)
                    nc.tensor.matmul(
                        ps[:, 0:nsz], lhsT=outT1[:, ts(qt_, 128)],
                        rhs=wo1[:, noff:noff + nsz], start=False, stop=True)
                    nc.vector.tensor_copy(
                        out=ot_tile[qb][:, sq, noff:noff + nsz],
                        in_=ps[:, 0:nsz])
            return emit

        def u_out(qb):
            # per-sq contiguous [128, D] transfers, split across the two
            # DMA-capable queues; collective stays on gpsimd
            def emit():
                for sq in range(4):
                    q = (nc.sync, nc.gpsimd)[sq % 2]
                    q.dma_start(out=out_part[ts(qb * 4 + sq, 128), :],
                                in_=ot_tile[qb][:, sq, :])
                if collective:
                    nc.gpsimd.collective_compute(
                        "ReduceScatter", mybir.AluOpType.add,
                        replica_groups=RGROUPS,
                        ins=[out_part[ts(qb, 512), :]],
                        outs=[out_rs[ts(qb, 128), :]])
                nc.gpsimd.dma_start(out=out_ext[ts(qb, 128), :],
                                    in_=(out_rs if collective
                                         else out_part)[ts(qb, 128), :])
            return emit

        def zip_emit(rounds, sec):
            """One ACT-gated score round, then the secondary units mapped
            to that slot; keys beyond len(rounds) form the tail."""
            for i, r in enumerate(rounds):
                r()
                for u in sec.get(i, ()):
                    u()
            for i in sorted(k for k in sec if k >= len(rounds)):
                for u in sec[i]:
                    u()

        def at(sec, i, u):
            sec.setdefault(i, []).append(u)

        # ---- prologue: k-proj block 0 + q-proj block 0 ----
        u_kproj(0)()
        u_qproj(0)()

        # ---- pipeline over q blocks ----
        # steady-state it(qb): rounds r0-15 sc01(qb,c=r), r16-23
        # sc2(qb,p=r-16). Secondary slots (see docstring):
        #   s0-6   pv1(qb-1) chunk pairs (2s+2, 2s+3)
        #   s c+2  pv0(qb) chunk c           (it0: s c+6, after vproj)
        #   s7     norm1(qb-1)  [+qp2/qp3 in it1/it2]
        #   s10-17 pv2(qb-1) pair p
        #   s18    norm0(qb), norm2(qb-1)
        #   s20-23 oproj(qb-1)
        #   tail   out(qb-1), pv1(qb) alloc + chunks 0,1
        for qb in range(NQB):
            rounds = ([u_sc01(qb, c) for c in range(NKC)] +
                      [u_sc2(qb, p) for p in range(NKC // 2)])
            sec = {}
            if qb == 0:
                for i, u in enumerate([u_kproj(1), u_g1proj(0), u_kproj(2),
                                       u_g1proj(1), u_kproj(3), u_g1proj(2),
                                       u_g1proj(3), u_qproj(1)]):
                    at(sec, i, u)
                for st in range(NKC):
                    at(sec, st + 6, u_vproj(st))
                for c in range(NKC):
                    at(sec, c + 7, u_pv(0, qb, (c,)))
                at(sec, 23, u_norm(0, qb))
            else:
                for s in range(7):
                    at(sec, s, u_pv(1, qb - 1, (2 * s + 2, 2 * s + 3)))
                at(sec, 7, u_norm(1, qb - 1))
                if qb == 1:
                    at(sec, 8, u_qproj(2))
                if qb == 2:
                    at(sec, 8, u_qproj(3))
                for c in range(NKC):
                    at(sec, c + 2, u_pv(0, qb, (c,)))
                for p in range(NKC // 2):
                    at(sec, 10 + p, u_pv(2, qb - 1, (2 * p, 2 * p + 1)))
                at(sec, 18, u_norm(0, qb))
                at(sec, 18, u_norm(2, qb - 1))
                for sq in range(4):
                    at(sec, 20 + sq, u_oproj(qb - 1, sq))
                at(sec, 24, u_out(qb - 1))
            # pv1(qb) first pair in the tail (before it(qb+1)'s round 0
            # overwrites e01 chunk 0)
            at(sec, 24, u_pv(1, qb, (0, 1)))
            zip_emit(rounds, sec)

        # ---- post: drain qb3 ----
        qb = NQB - 1
        for s in range(7):
            u_pv(1, qb, (2 * s + 2, 2 * s + 3))()
        u_norm(1, qb)()
        for p in range(NKC // 2):
            u_pv(2, qb, (2 * p, 2 * p + 1))()
        u_norm(2, qb)()
        for sq in range(4):
            u_oproj(qb, sq)()
        u_out(qb)()

    nc.compile()
    return nc


def _get_compiled():
    global _compiled
    if _compiled is None:
        _compiled = _build()
    return _compiled


def make_in_maps(q, k, v, Wq, bq, Wk, bk, Wv, bv, Wo, bo):
    bf = ml_dtypes.bfloat16
    in_maps = []
    for c in range(NCORES):
        b = c // GROUP
        g = c % GROUP
        cols = slice(g * HD, (g + 1) * HD)   # head-feature columns
        wo_aug = np.empty((HD + 1, D), np.float32)
        wo_aug[:HD] = Wo.T[cols.start:cols.stop, :]
        wo_aug[HD] = bo / GROUP              # summed GROUP times by the RS
        in_maps.append({
            "qt": np.ascontiguousarray(q[b].T).astype(bf),
            "kt": np.ascontiguousarray(k[b].T).astype(bf),
            "vt": np.ascontiguousarray(v[b].T).astype(bf),
            "wq": np.ascontiguousarray(Wq.T[:, cols]).astype(bf),
            "wk": np.ascontiguousarray(Wk.T[:, cols]).astype(bf),
            "wv": np.ascontiguousarray(Wv.T[:, cols]).astype(bf),
            "wo": wo_aug.astype(bf),
            "bq": np.ascontiguousarray(bq[cols].reshape(HD, 1)).astype(np.float32),
            "bk": np.ascontiguousarray(bk[cols].reshape(HD, 1)).astype(np.float32),
            "bv": np.ascontiguousarray(bv[cols].reshape(1, HD)).astype(np.float32),
        })
    return in_maps


def kernel(q, k, v, Wq, bq, Wk, bk, Wv, bv, Wo, bo):
    from concourse.bass_utils import run_bass_kernel_spmd

    q = np.asarray(q, np.float32)
    k = np.asarray(k, np.float32)
    v = np.asarray(v, np.float32)
    nc = _get_compiled()
    in_maps = make_in_maps(q, k, v,
                           np.asarray(Wq, np.float32), np.asarray(bq, np.float32),
                           np.asarray(Wk, np.float32), np.asarray(bk, np.float32),
                           np.asarray(Wv, np.float32), np.asarray(bv, np.float32),
                           np.asarray(Wo, np.float32), np.asarray(bo, np.float32))
    res = run_bass_kernel_spmd(nc, in_maps, list(range(NCORES))).results
    out = np.empty((B, S, D), np.float32)
    for c in range(NCORES):
        b = c // GROUP
        j = c % GROUP
        # chunked reduce-scatter: chunk g of core (b, j) holds batch-b
        # rows [512*g + 128*j, 512*g + 128*j + 128)
        chunks = res[c]["out"].reshape(GROUP, 128, D)
        for g in range(GROUP):
            out[b, 512 * g + 128 * j:512 * g + 128 * j + 128, :] = chunks[g]
    return out
